# revision 18
# baseline (speedup 1.0000x reference)
"""Trainium2 Bass kernel for additive (Show-Attend-Tell style) attention.

Reference computation (per batch b):
  e = enc.reshape(B, C, H*W).transpose(0,2,1)        # (B, P, C),  P = H*W
  att_enc = e @ W_enc + b_enc                        # (B, P, A)
  att_dec = dec @ W_dec + b_dec                      # (B, G, A)
  score[b,g,p] = sum_a w_alpha[a] * relu(att_enc[b,p,a] + att_dec[b,g,a]) (+ b_alpha)
  alpha = softmax_p(score)                           # (B, G, P)
  att_res = einsum('bgp,bpa->bga', alpha, att_enc)   # (B, G, A)
returns (att_res, alpha).  b_alpha shifts every score equally -> softmax
invariant -> ignored.

Sharding: data-parallel over batch, 4 batches per core on 8 cores.  The kernel
is PE-instruction bound, so the design minimizes PE instruction count and
streamed columns:
  - heavy matmuls in bf16 (1 cycle/column; plain fp32 runs at 1/4 rate) with
    fp32 PSUM accumulation; inputs are cast + relaid out host-side so every
    DMA descriptor is a long contiguous run.
  - att_enc^T (A on partitions, P free): 128 matmuls of N=392 (two batches
    share the moving operand); b_enc folds into the PSUM->SBUF evacuation
    (ACT Identity-with-bias).
  - dec^T comes pre-transposed from the host; att_dec is computed in (bg, a)
    layout (8 wide matmuls) and PE-transposed to (a, bg), folding b_dec into
    the evacuation (DVE add).
  - relu(att_enc^T + att_dec^T[:,g]): fused per-partition scalar add + max,
    split DVE (bf16 4x mode, ~2/3) / ACT (~1/3); batch pairs share one
    N=392 score matmul with a sparse (128, 20) stationary holding w_alpha
    in column g (zeros elsewhere accumulate harmlessly into other rows).
  - next batch-pair's att_enc matmuls and the (P, A) transposes interleave
    the relu-production-limited score matmuls as PE fillers.
  - softmax in fp32; att_res via paired alpha^T PE transposes (both batches
    in one transpose) against att_enc in (P, A) layout.
"""

import numpy as np

B, C, HW, G, D, A = 32, 2048, 196, 20, 1024, 512
NCORES = 8
BL = B // NCORES          # batches per core
KC = C // 128             # contraction tiles for enc proj
KD = D // 128             # contraction tiles for dec proj
MA = A // 128             # a-tiles
PT = [128, HW - 128]      # p-tile sizes (128 + 68)
BG = BL * G

_cache = {}
last_results = None


def _build(niter=1, lowp=True, parts=3):
    import contextlib
    import concourse.mybir as mybir
    import concourse.tile as tile
    from concourse import bacc
    from concourse.masks import make_identity

    f32 = mybir.dt.float32
    lp = mybir.dt.bfloat16 if lowp else mybir.dt.float32r
    AF = mybir.ActivationFunctionType
    ALU = mybir.AluOpType
    X = mybir.AxisListType.X

    nc = bacc.Bacc("TRN2", target_bir_lowering=False, debug=False)

    enc_d = nc.dram_tensor("enc", [128, BL * KC * HW], lp, kind="ExternalInput")
    decT_d = nc.dram_tensor("decT", [128, KD * BG], lp, kind="ExternalInput")
    wenc_d = nc.dram_tensor("wenc", [128, MA * KC * 128], lp,
                            kind="ExternalInput")
    wdec_d = nc.dram_tensor("wdec", [128, KD * A], lp, kind="ExternalInput")
    smalls_d = nc.dram_tensor("smalls", [3 * MA, 128], f32,
                              kind="ExternalInput")
    res_d = nc.dram_tensor("res", [BG, A], f32, kind="ExternalOutput")
    alpha_d = nc.dram_tensor("alpha", [BG, HW], f32, kind="ExternalOutput")

    with tile.TileContext(nc) as tc:
        loop_cm = (tc.For_i(0, niter, 1, hint_engines=tuple(mybir.EngineType))
                   if niter > 1 else contextlib.nullcontext())
        with (
            loop_cm,
            tc.tile_pool(name="const", bufs=1) as cpool,
            tc.tile_pool(name="big", bufs=1) as bpool,
            tc.tile_pool(name="rt", bufs=48) as rtpool,
            tc.tile_pool(name="small", bufs=2) as spool,
            tc.tile_pool(name="psum", bufs=1, space="PSUM") as ppool,
        ):
            ident = cpool.tile([128, 128], f32)
            make_identity(nc, ident[:])
            ident_l = cpool.tile([128, 128], lp)
            nc.vector.tensor_copy(ident_l[:], ident[:])

            # smalls: rows [w_alpha(4) | b_enc(4) | b_dec(4)] -> (128, 12)
            smalls_raw = cpool.tile([3 * MA, 128], f32)
            nc.sync.dma_start(smalls_raw[:], smalls_d.ap())
            sm_ps = ppool.tile([128, 3 * MA], f32, tag="acc", bufs=4)
            nc.tensor.transpose(sm_ps[:], smalls_raw[:],
                                ident[:3 * MA, :3 * MA])
            smalls_sb = cpool.tile([128, 3 * MA], f32)
            nc.scalar.copy(smalls_sb[:], sm_ps[:])
            wa_sb = smalls_sb[:, 0:MA]
            benc_sb = smalls_sb[:, MA:2 * MA]
            bdec_sb = smalls_sb[:, 2 * MA:3 * MA]

            # sparse score weights: wsp[:, g, m, :] = w_alpha[m-tile] at col g
            wsp_src = cpool.tile([128, G, MA, G], f32)
            nc.gpsimd.memset(wsp_src[:], 0.0)
            for g in range(G):
                nc.vector.tensor_copy(wsp_src[:, g, :, g], wa_sb)
            wsp = cpool.tile([128, G, MA, G], lp)
            nc.vector.tensor_copy(wsp[:], wsp_src[:])

            # dec-side loads on the gpsimd (SWDGE) ring; enc-side chunks
            # interleaved on the sync (HWDGE) ring so early matmuls can start
            wdec_sb = cpool.tile([128, KD, A], lp)
            nc.gpsimd.dma_start(wdec_sb[:], wdec_d.ap())
            decT_sb = cpool.tile([128, KD, BG], lp)
            nc.gpsimd.dma_start(decT_sb[:], decT_d.ap())
            wenc_sb = cpool.tile([128, MA, KC, 128], lp)
            enc_sb = bpool.tile([128, BL, KC, HW], lp)
            wenc_view = wenc_d.ap().rearrange("p (m r) -> p m r", m=MA)
            enc_view = enc_d.ap().rearrange("p (b r) -> p b r", b=BL)

            def wenc_chunk(c):
                nc.sync.dma_start(wenc_sb[:, c, :, :], wenc_view[:, c, :])

            def enc_chunk(b):
                nc.sync.dma_start(enc_sb[:, b, :, :], enc_view[:, b, :])

            wenc_chunk(0)
            enc_chunk(0)
            enc_chunk(1)
            for c in range(1, MA):
                wenc_chunk(c)
            enc_chunk(2)
            enc_chunk(3)

            # ---- att_dec in (bg, a) layout: KD wide matmuls ----
            ad_ps = ppool.tile([BG, A], f32, tag="acc", bufs=4)
            for kd in range(KD):
                nc.tensor.matmul(ad_ps[:], decT_sb[:, kd, :],
                                 wdec_sb[:, kd, :],
                                 start=(kd == 0), stop=(kd == KD - 1))
            adfull_sb = bpool.tile([BG, A], lp)
            nc.scalar.copy(adfull_sb[:], ad_ps[:])

            # transpose to (a, bg), folding b_dec into the evacuation
            adT_sb = bpool.tile([128, MA, BG], f32)
            for m in range(MA):
                t_ps = ppool.tile([128, BG], lp, tag="tp4", bufs=2)
                nc.tensor.transpose(t_ps[:],
                                    adfull_sb[:, m * 128:(m + 1) * 128],
                                    ident_l[:BG, :BG])
                nc.vector.tensor_scalar(
                    out=adT_sb[:, m, :], in0=t_ps[:],
                    scalar1=bdec_sb[:, m:m + 1], scalar2=None, op0=ALU.add)

            # ---- att_enc^T: b_enc folded into the ACT evacuation ----
            aeT_sb = bpool.tile([128, MA, BL, HW], lp)
            pa_sb = bpool.tile([128, 2, BL, A], lp)

            def emit_ae(bp):
                out = []
                state = {}

                def mk(m, k):
                    def f():
                        if k == 0:
                            state[m] = ppool.tile(
                                [128, 2 * HW], f32, tag="acc", bufs=4,
                                name=f"ae_ps{bp}_{m}")
                        nc.tensor.matmul(
                            state[m][:], wenc_sb[:, m, k, :],
                            enc_sb[:, 2 * bp:2 * bp + 2, k, :],
                            start=(k == 0), stop=(k == KC - 1))
                        if k == KC - 1:
                            nc.scalar.activation(
                                aeT_sb[:, m, 2 * bp:2 * bp + 2, :],
                                state[m][:], AF.Identity,
                                bias=benc_sb[:, m:m + 1])
                    return f
                for m in range(MA):
                    for k in range(KC):
                        out.append(mk(m, k))
                return out

            def emit_pa(b):
                out = []
                pa_view = pa_sb[:, :, b, :].rearrange(
                    "p pt (m c) -> p m pt c", c=128)

                def mk(mh):
                    def f():
                        t_ps4 = ppool.tile([128, 2, 2, 128], lp, tag="tp4",
                                           bufs=2, name=f"t_ps4_{b}_{mh}")
                        first = True
                        for mi in range(2):
                            m = 2 * mh + mi
                            for pt in range(2):
                                pl = PT[pt]
                                nc.tensor.matmul(
                                    t_ps4[:pl, mi, pt, :],
                                    aeT_sb[:, m, b, pt * 128:pt * 128 + pl],
                                    ident_l[:],
                                    is_transpose=True,
                                    start=first, stop=(mi == 1 and pt == 1))
                                first = False
                        nc.scalar.copy(pa_view[:, 2 * mh:2 * mh + 2, :, :],
                                       t_ps4[:])
                    return f
                for mh in range(2):
                    out.append(mk(mh))
                return out

            for f in emit_ae(0):
                f()

            res_stage = bpool.tile([G, BL, A], f32)
            alpha_stage = bpool.tile([G, BL, HW], f32)
            nc.gpsimd.memset(res_stage[:], 0.0)
            nc.gpsimd.memset(alpha_stage[:], 0.0)
            nrelu = 0
            for bp in range(BL // 2 if parts >= 2 else 0):
                fillers = []
                if bp + 1 < BL // 2:
                    fillers += emit_ae(bp + 1)
                fillers += emit_pa(2 * bp)
                fillers += emit_pa(2 * bp + 1)
                nscore = G * MA
                stride = max(1, nscore // max(1, len(fillers)))

                score_ps = ppool.tile([G, 2, HW], f32, tag="sc", bufs=2)
                nmm = 0
                for m in range(MA):
                    for g in range(G):
                        rt = rtpool.tile([128, 2, HW], lp)
                        for i in range(2):
                            b = 2 * bp + i
                            col = b * G + g
                            if nrelu % 3 < 2:
                                nc.vector.tensor_scalar(
                                    out=rt[:, i, :],
                                    in0=aeT_sb[:, m, b, :],
                                    scalar1=adT_sb[:, m, col:col + 1],
                                    scalar2=0.0, op0=ALU.add, op1=ALU.max)
                            else:
                                nc.scalar.activation(
                                    rt[:, i, :], aeT_sb[:, m, b, :],
                                    AF.Relu, bias=adT_sb[:, m, col:col + 1])
                            nrelu += 1
                        nc.tensor.matmul(
                            score_ps[:], wsp[:, g, m, :], rt[:],
                            start=(m == 0 and g == 0),
                            stop=(m == MA - 1 and g == G - 1))
                        nmm += 1
                        if nmm % stride == 0:
                            nfill = 1 if len(fillers) < nscore else 2
                            for _ in range(nfill):
                                if fillers:
                                    fillers.pop(0)()
                for f in fillers:
                    f()

                if parts < 3:
                    continue
                negmax = spool.tile([G, 2], f32, tag="negmax")
                nc.vector.tensor_reduce(out=negmax[:], in_=score_ps[:],
                                        axis=X, op=ALU.max, negate=True)
                alpha_l = spool.tile([64, HW], lp, tag="alphal")
                for i in range(2):
                    b = 2 * bp + i
                    exp_sb = spool.tile([G, HW], f32, tag="exp")
                    nc.scalar.activation(exp_sb[:], score_ps[:, i, :], AF.Exp,
                                         bias=negmax[:, i:i + 1])
                    ssum = spool.tile([G, 1], f32, tag="ssum")
                    nc.vector.reduce_sum(out=ssum[:], in_=exp_sb[:], axis=X)
                    recip = spool.tile([G, 1], f32, tag="recip")
                    nc.vector.reciprocal(recip[:], ssum[:])
                    nc.vector.tensor_scalar(
                        out=alpha_stage[:, b, :], in0=exp_sb[:],
                        scalar1=recip[:], scalar2=None, op0=ALU.mult)
                    nc.vector.tensor_scalar(
                        out=alpha_l[32 * i:32 * i + G, :], in0=exp_sb[:],
                        scalar1=recip[:], scalar2=None, op0=ALU.mult)

                # paired alpha^T transposes for both batches of the pair
                at_sb = spool.tile([128, 2, 64], lp, tag="at")
                t_ps2 = ppool.tile([128, 2, 64], lp, tag="tp4", bufs=2)
                for pt in range(2):
                    pl = PT[pt]
                    nc.tensor.matmul(
                        t_ps2[:pl, pt, :],
                        alpha_l[:, pt * 128:pt * 128 + pl],
                        ident_l[:64, :64],
                        is_transpose=True,
                        start=(pt == 0), stop=(pt == 1))
                nc.scalar.copy(at_sb[:], t_ps2[:])

                for i in range(2):
                    b = 2 * bp + i
                    ar_ps = ppool.tile([G, A], f32, tag="sc", bufs=2)
                    for pt in range(2):
                        pl = PT[pt]
                        nc.tensor.matmul(
                            ar_ps[:], at_sb[:pl, pt, 32 * i:32 * i + G],
                            pa_sb[:pl, pt, b, :],
                            start=(pt == 0), stop=(pt == 1))
                    nc.scalar.copy(res_stage[:, b, :], ar_ps[:])

            nc.sync.dma_start(
                alpha_d.ap().rearrange("(b g) n -> g b n", g=G),
                alpha_stage[:])
            nc.sync.dma_start(
                res_d.ap().rearrange("(b g) n -> g b n", g=G), res_stage[:])

    nc.compile()
    return nc


def _prep_in_maps(enc, dec, W_enc, b_enc, W_dec, b_dec, w_alpha,
                  _lowp=True):
    import ml_dtypes
    lpnp = ml_dtypes.bfloat16 if _lowp else np.float32
    # partition-major relayouts so every DMA descriptor is a long run
    enc = np.ascontiguousarray(
        enc.reshape(B, C, HW).astype(lpnp).reshape(B, KC, 128, HW)
        .transpose(2, 0, 1, 3))                       # (128, B, KC, HW)
    # decT[p, b, kd, g] = dec[b, g, kd*128+p]
    decT = np.ascontiguousarray(
        dec.astype(lpnp).reshape(B, G, KD, 128).transpose(3, 0, 2, 1))
    smalls = np.ascontiguousarray(np.concatenate([
        w_alpha.reshape(MA, 128), b_enc.reshape(MA, 128),
        b_dec.reshape(MA, 128)], axis=0), dtype=np.float32)
    shared = {
        "wenc": np.ascontiguousarray(
            W_enc.astype(lpnp).reshape(KC, 128, MA, 128)
            .transpose(1, 2, 0, 3).reshape(128, MA * KC * 128)),
        "wdec": np.ascontiguousarray(
            W_dec.astype(lpnp).reshape(KD, 128, A).transpose(1, 0, 2)
            .reshape(128, KD * A)),
        "smalls": smalls,
    }
    in_maps = []
    for c in range(NCORES):
        bs = slice(c * BL, (c + 1) * BL)
        in_maps.append(dict(
            enc=enc[:, bs].reshape(128, BL * KC * HW),
            decT=np.ascontiguousarray(
                decT[:, bs].transpose(0, 2, 1, 3)).reshape(128, KD * BG),
            **shared,
        ))
    return in_maps


def kernel(enc, dec, W_enc, b_enc, W_dec, b_dec, w_alpha, b_alpha,
           _trace=False, _niter=1, _lowp=True, _parts=3, **_unused):
    global last_results
    from concourse.bass_utils import run_bass_kernel_spmd

    key = ("nc", _niter, _lowp, _parts)
    if key not in _cache:
        _cache[key] = _build(_niter, _lowp, _parts)
    nc = _cache[key]

    in_maps = _prep_in_maps(enc, dec, W_enc, b_enc, W_dec, b_dec, w_alpha,
                            _lowp)
    res = run_bass_kernel_spmd(nc, in_maps, core_ids=list(range(NCORES)),
                               trace=_trace)
    last_results = res

    att_res = np.concatenate(
        [res.results[c]["res"].reshape(BL, G, A) for c in range(NCORES)])
    alpha = np.concatenate(
        [res.results[c]["alpha"].reshape(BL, G, HW) for c in range(NCORES)])
    return att_res, alpha
